# revision 1
# baseline (speedup 1.0000x reference)
"""CombinedAttentionProcessor kernel for 8 Trainium2 NeuronCores.

Problem: B=2, S=4096, C=640, H=8 heads, D=80 head_dim.
    q/k/v = hs @ W{q,k,v}.T ; per-(b,h): softmax(q k^T / sqrt(D)) v ;
    out = attn @ Wo.T + bo + residual.

Sharding: 16 (batch, head) groups -> 2 per core (batch-parallel over B,
head-parallel over H). Each core computes its 2 heads' full attention and a
partial output projection [S, C]; the host sums the 4 partials per batch and
adds bias + residual.

Per-core dataflow (matmuls in fp32r = full PE rate at moving dim >= 256;
probabilities and V in bf16):
  Phase A: load hsT [C, S]; project qT/kT [D, S] (d-major) and v [S, D]
           (natural, with a ones column at position 96 for the softmax
           row-sum; the V moving operand is host-padded to 256 so the fp32r
           matmul stays in its 1-cycle/row regime).
  Phase B: per head, per 512-query chunk: scoresT tiles [128 keys, 512 q]
           on PE; exp(scale*x) on ScalarE (PSUM->SBUF, bf16); AV matmul
           accumulates out_avT [97, 512] over the 32 key tiles -- row 96 is
           the softmax denominator. Normalize with reciprocal + a broadcast
           matmul (ones[128,80] row-0 one-hot) + DVE multiply.
  Phase C: output projection per 128-query tile, accumulated in PSUM over
           both heads, staged into SBUF quarter-buffers and written with 4
           large DMAs.

All DRAM I/O uses host-prepared partition-major layouts so each DMA is 128
contiguous per-partition descriptors (the DMA sequencer's per-descriptor
issue cost would otherwise dominate). fp32r matmuls admit only ONE sync
wait; dummy matmuls right after the input DMAs make PE observe every
DMA-queue semaphore once, and engine assignment keeps every real matmul's
unobserved waits on a single semaphore.
"""
import sys

if "/opt/trn_rl_repo" not in sys.path:
    sys.path.insert(0, "/opt/trn_rl_repo")

import numpy as np

B, S, C = 2, 4096, 640
H, D = 8, 80
HPC = 2          # heads per core
NCORES = 8
KC = C // 128    # 5 contraction tiles over C
WVN = 256        # v-projection moving width (160 data + zero pad)
WON = 768        # wo moving width (640 data + zero pad; keeps fp32r at 1 cyc/row)
SCALE = 1.0 / float(np.sqrt(D))

_NC_CACHE = {}


def build_nc(s=S):
    import concourse.bacc as bacc
    import concourse.mybir as mybir
    import concourse.tile as tile
    from concourse.tile import add_dep_helper

    f32 = mybir.dt.float32
    f32r = mybir.dt.float32r
    bf16 = mybir.dt.bfloat16

    njt = s // 128   # key tiles
    nit = s // 128   # output i-tiles
    nch = s // 512   # query chunks
    qsize = max(1, nit // 16)  # i-tiles per output staging buffer
    assert s % 512 == 0 and (s // 128) % 4 == 0

    nc = bacc.Bacc("TRN2", target_bir_lowering=False, debug=False,
                   num_devices=NCORES)

    # all inputs partition-major, host-prepared (incl. padding)
    hsT = nc.dram_tensor("hsT", [128, KC * s], f32r, kind="ExternalInput")
    wq = nc.dram_tensor("wq", [128, KC * HPC * D], f32r,
                        kind="ExternalInput")
    wk = nc.dram_tensor("wk", [128, KC * HPC * D], f32r,
                        kind="ExternalInput")
    wv = nc.dram_tensor("wv", [128, KC * WVN], f32r, kind="ExternalInput")
    wo = nc.dram_tensor("wo", [128, HPC * WON], f32r,
                        kind="ExternalInput")
    o_dram = nc.dram_tensor("o", [128, nit * C], f32, kind="ExternalOutput")

    with tile.TileContext(nc) as tc:
        with (
            tc.tile_pool(name="persist", bufs=1) as pp,
            tc.tile_pool(name="ppt", bufs=5) as ppt,
            tc.tile_pool(name="pbcs", bufs=2) as pbcs,
            tc.tile_pool(name="psc_ps", bufs=3, space="PSUM") as psc,
            tc.tile_pool(name="pav_ps", bufs=1, space="PSUM") as pav,
            tc.tile_pool(name="pbc_ps", bufs=1, space="PSUM") as pbc,
        ):
            # ---- persistent tiles ----
            # qT/kT in bf16: halves SBUF so the hsT staging pool can stay
            # open through the whole head-0 attention phase (head-1
            # projections are interleaved into it as PE filler work)
            qT = [pp.tile([128, s], bf16, name=f"qT{h}") for h in range(HPC)]
            kT = [pp.tile([128, s], bf16, name=f"kT{h}") for h in range(HPC)]
            # v tiles per key-tile, per-head stride 97:
            # [head data (80) | zero pad (16) | one] -- ones at 96 so the AV
            # row sum lands on a 32-aligned PSUM partition
            VS = 97
            v_sb = pp.tile([128, njt, 2 * VS], bf16, name="v_sb")
            wo_sb = pp.tile([128, HPC, WON], f32r, name="wo_sb")
            avn = [pp.tile([128, s], f32r, name=f"avn{h}")
                   for h in range(HPC)]
            recip_sb = pp.tile([128, 512], f32r, name="recip_sb")
            ones_sb = pp.tile([128, D], f32r, name="ones_sb")

            # f32r tiles can't be Memset; fill via DVE copy from a broadcast
            # f32 source (the engine cast satisfies the fp32r rounding rule).
            # All on DVE so matmul waits merge on one semaphore.
            zsrc = pp.tile([128, 8], f32, name="zsrc")
            osrc = pp.tile([128, 8], f32, name="osrc")
            nc.vector.memset(zsrc[:], 0.0)
            nc.vector.memset(osrc[:], 1.0)

            def zfill(dst2d):
                nc.vector.tensor_copy(
                    dst2d, zsrc[:dst2d.shape[0], 0:1].broadcast_to(
                        dst2d.shape))

            nc.vector.memset(v_sb[:, :, :], 0.0)
            nc.vector.memset(v_sb[:, :, VS - 1], 1.0)
            nc.vector.memset(v_sb[:, :, 2 * VS - 1], 1.0)
            for h in range(HPC):
                nc.vector.memset(kT[h][:, :], 0.0)
                nc.vector.memset(qT[h][:, :], 0.0)
                zfill(avn[h][:, :])
            zfill(recip_sb[:, :])
            zfill(ones_sb[:, :])
            nc.vector.tensor_copy(
                ones_sb[0:1, :], osrc[0:1, 0:1].broadcast_to([1, D]))

            mul_prev = [None]

            def chunk_body(h, i8, filler_hook=None):
                """Attention for one (head, 512-query chunk)."""
                i0 = i8 * 512
                if mul_prev[0] is not None:
                    # absorb the av-slot WAR (DVE) on a dummy so the first
                    # AV matmul below carries only the ACT wait
                    dum2 = pbc.tile([8, 8], f32, name="dum2", tag="bcslot")
                    dmm = nc.tensor.matmul(
                        dum2[:], ones_sb[0:1, 0:8], ones_sb[0:1, 0:8],
                        start=True, stop=True, skip_group_check=True)
                    add_dep_helper(dmm.ins, mul_prev[0].ins,
                                   reason="absorb av WAR on PE")
                av = pav.tile([VS, 512], f32, name="av_ps")
                for jg in range(njt // 2):
                    if filler_hook is not None and (
                            filler_hook.__name__ == "urgent_hook"
                            or jg % 4 == 3):
                        filler_hook()
                    sc = psc.tile([128, 1024], f32, name="sc_ps", tag="scslot")
                    for jj in range(2):
                        j = 2 * jg + jj
                        nc.tensor.matmul(
                            sc[:, jj * 512:(jj + 1) * 512],
                            kT[h][:, j * 128:(j + 1) * 128],
                            qT[h][:, i0:i0 + 512],
                            start=True, stop=True,
                        )
                    pt = ppt.tile([128, 1024], bf16, name="pt")
                    nc.scalar.activation(
                        out=pt[:], in_=sc[:],
                        func=mybir.ActivationFunctionType.Exp,
                        scale=SCALE,
                    )
                    for jj in range(2):
                        j = 2 * jg + jj
                        nc.tensor.matmul(
                            av[:],
                            v_sb[:, j, h * VS:(h + 1) * VS],
                            pt[:, jj * 512:(jj + 1) * 512],
                            start=(j == 0), stop=(j == njt - 1),
                        )
                # normalize: avn = av[0:D] * (1 / rowsum) broadcast
                with nc.allow_low_precision(
                        reason="fp32r recip feeds broadcast matmul"):
                    nc.vector.reciprocal(recip_sb[0:1, :],
                                         av[VS - 1:VS, :])
                bc = pbc.tile([D, 512], f32, name="bc_ps", tag="bcslot")
                nc.tensor.matmul(bc[:], ones_sb[:], recip_sb[:],
                                 start=True, stop=True)
                av_sb2 = pbcs.tile([D, 512], f32, name="av_sb2")
                nc.vector.tensor_copy(av_sb2[:], av[0:D, :])
                mul_prev[0] = nc.vector.tensor_mul(
                    avn[h][0:D, i0:i0 + 512], av_sb2[:], bc[:])

            # ============ Phase A + head-0 attention (hsT resident) =======
            with (
                tc.tile_pool(name="pA", bufs=1) as pA,
            ):
                hsT_sb = pA.tile([128, KC, s], f32r, name="hsT_sb")
                wq_sb = pA.tile([128, KC, HPC * D], f32r, name="wq_sb")
                wk_sb = pA.tile([128, KC, HPC * D], f32r, name="wk_sb")
                wv_sb = pA.tile([128, KC, WVN], f32r, name="wv_sb")

                # weights first (small, needed by the first matmuls),
                # then hsT in two column-half batches per kc chunk so the
                # first half of the projections can start at ~half DMA time
                nc.sync.dma_start(
                    wk_sb.rearrange("p a b -> p (a b)"), wk[:, :])
                nc.sync.dma_start(
                    wv_sb.rearrange("p a b -> p (a b)"), wv[:, :])
                nc.sync.dma_start(
                    wq_sb.rearrange("p a b -> p (a b)"), wq[:, :])
                nc.sync.dma_start(
                    wo_sb.rearrange("p a b -> p (a b)"), wo[:, :])
                hh = s // 2
                qq = s // 4
                for cb in range(4):
                    for kc in range(KC):
                        nc.sync.dma_start(
                            hsT_sb[:, kc, cb * qq:(cb + 1) * qq],
                            hsT[:, kc * s + cb * qq:kc * s + cb * qq + qq])

                # dummy matmuls: make PE observe every DMA-queue semaphore
                # (fp32r matmuls can carry only one sync wait each);
                # batch-2 dummies are emitted after the first-half work below
                dum = pbc.tile([8, 8], f32, name="dum", tag="bcslot")
                for src in ([wq_sb[0:1, 0, 0:8], wk_sb[0:1, 0, 0:8],
                             wv_sb[0:1, 0, 0:8], wo_sb[0:1, 0, 0:8]] +
                            [hsT_sb[0:1, kc, 0:8] for kc in range(KC)] +
                            [hsT_sb[0:1, kc, qq:qq + 8]
                             for kc in range(KC)]):
                    nc.tensor.matmul(dum[:], src, src, start=True, stop=True,
                                     skip_group_check=True)

                def emit_qk_chunk(h, w_sb, dst, iq):
                    ps = psc.tile([D, 512], f32, name="qk_ps", tag="scslot")
                    for kc in range(KC):
                        nc.tensor.matmul(
                            ps[:],
                            w_sb[:, kc, h * D:(h + 1) * D],
                            hsT_sb[:, kc, iq * 512:(iq + 1) * 512],
                            start=(kc == 0), stop=(kc == KC - 1),
                        )
                    cp = nc.vector.tensor_copy(
                        dst[0:D, iq * 512:(iq + 1) * 512], ps[:])
                    del cp

                def emit_v_tile(jt):
                    ps = psc.tile([128, WVN], f32, name="v_ps", tag="scslot")
                    for kc in range(KC):
                        nc.tensor.matmul(
                            ps[:],
                            hsT_sb[:, kc, jt * 128:(jt + 1) * 128],
                            wv_sb[:, kc, :],
                            start=(kc == 0), stop=(kc == KC - 1),
                        )
                    c0 = nc.vector.tensor_copy(v_sb[:, jt, 0:D], ps[:, 0:D])
                    c1 = nc.vector.tensor_copy(v_sb[:, jt, VS:VS + D],
                                               ps[:, D:2 * D])
                    del c0, c1

                # minimal prefix for head-0 attention: first halves of
                # kT[0] and v (covering key tiles 0..njt/2-1) + qT[0] i0;
                # the second halves are emitted as high-rate fillers inside
                # chunk 0's attention loop (they stay ahead of consumption)
                for iq in range(nch // 2):
                    emit_qk_chunk(0, wk_sb, kT[0], iq)
                for jt in range(njt // 2):
                    emit_v_tile(jt)
                for kc in range(KC):
                    for off in (hh, hh + qq):
                        src2 = hsT_sb[0:1, kc, off:off + 8]
                        nc.tensor.matmul(dum[:], src2, src2, start=True,
                                         stop=True, skip_group_check=True)
                emit_qk_chunk(0, wq_sb, qT[0], 0)

                # remaining projections become PE filler work inside the
                # head-0 attention loop (ACT-paced -> PE has slack there)
                # urgent fillers: second halves of kT[0] / v, interleaved
                # so supply stays ahead of the chunk-0 attention loop's
                # consumption (kT iq covers 4 key tiles, each jg eats 2)
                urgent = []
                vj = njt // 2
                for iq in range(nch // 2, nch):
                    urgent.append(("k0", iq))
                    for _ in range(4):
                        if vj < njt:
                            urgent.append(("v", vj))
                            vj += 1
                while vj < njt:
                    urgent.append(("v", vj))
                    vj += 1
                fillers = (
                    [(0, "q", iq) for iq in range(1, nch)] +
                    [(1, "k", iq) for iq in range(nch)] +
                    [(1, "q", iq) for iq in range(nch)]
                )
                fq = list(fillers)
                emitted = {(0, "q", 0)}

                def filler_hook(n=1):
                    for _ in range(n):
                        if urgent:
                            kind, idx = urgent.pop(0)
                            if kind == "k0":
                                emit_qk_chunk(0, wk_sb, kT[0], idx)
                            else:
                                emit_v_tile(idx)
                        elif fq:
                            h2, t2, iq2 = fq.pop(0)
                            w2 = wq_sb if t2 == "q" else wk_sb
                            d2 = qT[h2] if t2 == "q" else kT[h2]
                            emit_qk_chunk(h2, w2, d2, iq2)
                            emitted.add((h2, t2, iq2))

                def urgent_hook():
                    # 2 per jg: outpaces consumption (2 v-tiles + 0.5 kT
                    # groups per jg, starting 8 jg in) without starving
                    # ScalarE behind a PE filler burst
                    filler_hook(2)

                for i8 in range(nch):
                    # dependency order is EMISSION order: this chunk's qT
                    # slice and all urgent work must precede its consumers
                    if i8 > 0:
                        while urgent:
                            filler_hook()
                    while (0, "q", i8) not in emitted:
                        filler_hook()
                    chunk_body(0, i8,
                               urgent_hook if i8 == 0 else filler_hook)
                while fq or urgent:
                    filler_hook()

            # ============ head-1 attention + output projection ============
            with (
                tc.tile_pool(name="pobuf", bufs=2) as pobuf,
            ):
                o_state = {"buf": None}
                cq = []   # deferred Phase-C i-tile indices

                def emit_c_tile(g):
                    if g % qsize == 0:
                        o_state["buf"] = pobuf.tile([128, qsize, C], f32,
                                                    name="o_buf")
                    o_buf = o_state["buf"]
                    t0 = g * 128
                    o_ps = psc.tile([128, WON], f32, name="o_ps", tag="scslot")
                    for n0, n1 in ((0, 512), (512, WON)):
                        for h in range(HPC):
                            nc.tensor.matmul(
                                o_ps[:, n0:n1],
                                avn[h][:, t0:t0 + 128],
                                wo_sb[:, h, n0:n1],
                                start=(h == 0), stop=(h == HPC - 1),
                            )
                    nc.vector.tensor_copy(o_buf[:, g % qsize, :], o_ps[:, 0:C])
                    if g % qsize == qsize - 1:
                        q = g // qsize
                        nc.sync.dma_start(
                            o_dram[:, q * qsize * C:(q + 1) * qsize * C],
                            o_buf.rearrange("p a b -> p (a b)"),
                        )

                def c_hook():
                    if cq:
                        emit_c_tile(cq.pop(0))

                for i8 in range(nch):
                    # Phase C of the previous chunk interleaves into this
                    # chunk's attention loop (keeps ACT fed at boundaries)
                    chunk_body(1, i8, c_hook if cq else None)
                    while cq:
                        c_hook()
                    cq.extend(range(i8 * 4, i8 * 4 + 4))
                while cq:
                    c_hook()

    nc.compile()
    return nc


def _get_nc(s=S):
    if s not in _NC_CACHE:
        _NC_CACHE[s] = build_nc(s)
    return _NC_CACHE[s]


def _pmajor(a, width):
    """[KC*128, width] -> partition-major [128, KC*width]."""
    kc = a.shape[0] // 128
    return np.ascontiguousarray(
        a.reshape(kc, 128, width).transpose(1, 0, 2).reshape(128, kc * width))


def make_in_maps(hidden_states, Wq, Wk, Wv, Wo, s=S):
    """Shard full inputs into 8 per-core input dicts (partition-major)."""
    hs = np.asarray(hidden_states, dtype=np.float32)
    Wq = np.asarray(Wq, dtype=np.float32)
    Wk = np.asarray(Wk, dtype=np.float32)
    Wv = np.asarray(Wv, dtype=np.float32)
    Wo = np.asarray(Wo, dtype=np.float32)
    hsT = [_pmajor(np.ascontiguousarray(hs[b].T), s) for b in range(B)]
    in_maps = []
    for c in range(NCORES):
        b, hp = divmod(c, NCORES // B)
        rows = slice(HPC * D * hp, HPC * D * (hp + 1))
        wv_t = np.ascontiguousarray(Wv[rows, :].T)          # [C, 160]
        wv_pad = np.zeros((C, WVN), np.float32)
        wv_pad[:, :HPC * D] = wv_t
        wo_t = np.ascontiguousarray(Wo[:, rows].T)          # [160, C]
        wo_pad = np.zeros((HPC, 128, WON), np.float32)
        wo_pad[:, :D, :C] = wo_t.reshape(HPC, D, C)
        wo_pm = np.ascontiguousarray(
            wo_pad.transpose(1, 0, 2).reshape(128, HPC * WON))
        in_maps.append({
            "hsT": hsT[b],
            "wq": _pmajor(np.ascontiguousarray(Wq[rows, :].T), HPC * D),
            "wk": _pmajor(np.ascontiguousarray(Wk[rows, :].T), HPC * D),
            "wv": _pmajor(wv_pad, WVN),
            "wo": wo_pm,
        })
    return in_maps


def unpermute_o(o_core, s=S):
    """[128, (s/128)*C] partition-major -> [s, C]."""
    nit = s // 128
    return o_core.reshape(128, nit, C).transpose(1, 0, 2).reshape(s, C)


def assemble(results, hidden_states, bo):
    hs = np.asarray(hidden_states, dtype=np.float32)
    bo = np.asarray(bo, dtype=np.float32)
    out = np.empty((B, S, C), dtype=np.float32)
    ncb = NCORES // B
    for b in range(B):
        acc = unpermute_o(results[b * ncb]["o"]).astype(np.float64)
        for k in range(1, ncb):
            acc = acc + unpermute_o(results[b * ncb + k]["o"])
        out[b] = (acc + bo[None, :]).astype(np.float32) + hs[b]
    return out


def kernel(hidden_states, Wq, Wk, Wv, Wo, bo):
    from concourse.bass_utils import run_bass_kernel_spmd

    nc = _get_nc(S)
    in_maps = make_in_maps(hidden_states, Wq, Wk, Wv, Wo)
    res = run_bass_kernel_spmd(nc, in_maps, core_ids=list(range(NCORES)))
    return assemble(res.results, hidden_states, bo)

